# revision 13
# baseline (speedup 1.0000x reference)
"""LoRA linear kernel for 8 TRN2 NeuronCores.

Computes out = x @ (base_weight + SCALE * lora_B @ lora_A).T + bias
for x [4, 2048, 4096], base_weight [4096, 4096], rank 8.

Strategy (v5):
  - The LoRA fold W' = W + SCALE*(B@A) is 0.1% of the flops -> computed on
    the HOST in numpy. The device kernel is a pure GEMM + bias.
  - Sharding: 2 token-halves x 4 d_out-quarters = 8 cores (tensor-parallel
    on d_out per the hint, plus a token split that cuts per-core x traffic).
  - fp8 3-term matmul ('f8x3'): the PE runs e4m3 DoubleRow matmuls at 2x
    rate (0.5 cyc/row).  Pure fp8 quantization error is ~3.7e-2 rel L2
    (fails the 2e-2 gate), so x and W' are each split into an fp8 high +
    fp8 residual part at the SAME power-of-2 scale:
        x*sx ~ xh + xl,  W'*sw ~ Wh + Wl
        out = [xh@Wh + xl@Wh + xh@Wl] / (sx*sw) + bias    (~1.1e-3 rel L2)
    Equal scales let all 3 terms accumulate into one fp32 PSUM bank; the
    1/(sx*sw) rescale + bias ride the flush as one DVE scalar_tensor_tensor.
    3 terms x 0.5 cyc/row = 75% of the fp16/f32r PE cycles.
  - Per core: Wh/Wl cached in SBUF as fp8 [128, 4, O_CORE] groups (8 MB).
    Main loop per 128-token tile: two 0.5 MB x DMAs (hi/lo, [128k, 32kt,
    128tok] pre-tiled on host), 3x16x2 accumulating DoubleRow matmuls into
    2 [128, 512] PSUM banks (x k-pair stationary, W' k-pair moving), DVE
    rescale+bias into fp16 out tile, 0.25 MB out DMA (host upcasts).
  - x loads issue on Sync, out stores on GpSimd (separate queues so the
    next iteration's x prefetch isn't serialized behind out-store waits).
  - The k-major INTRO interleave (hides the W'-stream ramp) runs ONCE
    before the repeat loop; repeat iterations run the plain token-major
    loop.

Sustained PE clock is ~2.0 GHz (2.4 GHz burst): fp16 floor is
2048 matmuls x 256 ns = 524 us/pass; f8x3 targets ~0.75 of that.
"""
import sys

if '/opt/trn_rl_repo' not in sys.path:
    sys.path.insert(0, '/opt/trn_rl_repo')

from contextlib import ExitStack

import numpy as np
import ml_dtypes

import concourse.bacc as bacc
import concourse.mybir as mybir
import concourse.tile as tile
from concourse.bass_utils import run_bass_kernel_spmd

SCALE = 16.0 / 8.0  # alpha / rank

MODE = "f16"        # 'f8x3' (fp8 DoubleRow 3-term) or 'f16'
SX = 16.0           # fp8 quantization scale for x  (|x*SX|  < 240)
SW = 1024.0         # fp8 quantization scale for W' (|W'*SW| < 240)
F8 = ml_dtypes.float8_e4m3

P = 128
K = 4096           # d_in (contraction)
KT = K // P        # 32 k-tiles
D_OUT = 4096
B, S = 4, 2048
T_FULL = B * S     # 8192 tokens

R_SPLIT, C_SPLIT = 2, 4
N_CORES = R_SPLIT * C_SPLIT
T_CORE = T_FULL // R_SPLIT
TT = T_CORE // P              # token tiles/core
O_CORE = D_OUT // C_SPLIT
OC = O_CORE // 512            # o-chunks of 512
WPACK = 4                     # k-tiles per W' SBUF tile

_nc_cache = {}


def build_nc(repeat=1):
    """Build the per-core Bass program. `repeat` re-runs the main loop that
    many times (first pass with the INTRO ramp, the rest in a hardware
    loop; identical results; used for slope-based HW timing)."""
    key = (MODE, repeat)
    if key in _nc_cache:
        return _nc_cache[key]
    f32 = mybir.dt.float32
    f16 = mybir.dt.float16
    f8 = mybir.dt.float8e4
    fp8 = MODE == "f8x3"
    mm_dt = f8 if fp8 else f16
    DR = mybir.MatmulPerfMode.DoubleRow if fp8 else None

    nc = bacc.Bacc(None, target_bir_lowering=False)
    # x blocks: [t_tile, p(k-within-tile), kt, j(token-within-tile)]
    xbh = nc.dram_tensor("xbh", [TT, P, KT, P], mm_dt, kind="ExternalInput")
    wth = nc.dram_tensor("wth", [KT // WPACK, P, WPACK, O_CORE], mm_dt,
                         kind="ExternalInput")
    if fp8:
        xbl = nc.dram_tensor("xbl", [TT, P, KT, P], f8, kind="ExternalInput")
        wtl = nc.dram_tensor("wtl", [KT // WPACK, P, WPACK, O_CORE], f8,
                             kind="ExternalInput")
    biasb = nc.dram_tensor("biasb", [P, O_CORE], f32, kind="ExternalInput")
    out = nc.dram_tensor("out", [T_CORE, O_CORE], f16, kind="ExternalOutput")

    with ExitStack() as ctx:
        tc = ctx.enter_context(tile.TileContext(nc))
        wpool = ctx.enter_context(tc.tile_pool(name="wpool", bufs=1))
        cpool = ctx.enter_context(tc.tile_pool(name="cpool", bufs=1))
        xpool = ctx.enter_context(tc.tile_pool(name="xpool", bufs=4))
        opool = ctx.enter_context(tc.tile_pool(name="opool", bufs=3))
        pspool = ctx.enter_context(tc.tile_pool(name="ps", bufs=4,
                                                space="PSUM"))

        # ---- constants / W' stream ----
        bias_t = cpool.tile([P, O_CORE], f32, tag="bias")
        nc.sync.dma_start(bias_t[:], biasb[:])

        # W' group tiles; interleave hi/lo groups so the k-major INTRO can
        # consume them in stream order.
        wh_g, wl_g = [], []
        for g in range(KT // WPACK):
            w_g = wpool.tile([P, WPACK, O_CORE], mm_dt, tag=f"wgh{g}")
            nc.sync.dma_start(w_g[:], wth[g])
            wh_g.append(w_g)
            if fp8:
                l_g = wpool.tile([P, WPACK, O_CORE], f8, tag=f"wgl{g}")
                nc.sync.dma_start(l_g[:], wtl[g])
                wl_g.append(l_g)

        # ---- main loop ----
        def load_x(tt):
            xh = xpool.tile([P, KT, P], mm_dt, name=f"xh_{tt}", tag="xh")
            nc.sync.dma_start(xh[:], xbh[tt])
            if not fp8:
                return xh, None
            xl = xpool.tile([P, KT, P], f8, name=f"xl_{tt}", tag="xl")
            nc.sync.dma_start(xl[:], xbl[tt])
            return xh, xl

        def alloc_ps(tt):
            return [pspool.tile([P, 512], f32, tag=f"ps{oc}",
                                name=f"ps_{tt}_{oc}")
                    for oc in range(OC)]

        def flush(tt, pss):
            o_t = opool.tile([P, O_CORE], f16, name=f"ot_{tt}", tag="ot")
            for oc in range(OC):
                sl = slice(oc * 512, (oc + 1) * 512)
                if fp8:
                    nc.vector.scalar_tensor_tensor(
                        o_t[:, sl], pss[oc][:], 1.0 / (SX * SW),
                        bias_t[:, sl],
                        op0=mybir.AluOpType.mult, op1=mybir.AluOpType.add)
                else:
                    nc.vector.tensor_add(o_t[:, sl], pss[oc][:],
                                         bias_t[:, sl])
            nc.gpsimd.dma_start(out[tt * P:(tt + 1) * P, :], o_t[:])

        def mm_steps(xt, pss, kstart, kstop):
            """Emit the matmuls for one k-step of one token tile.
            fp8: k-step = pair of k-tiles (DoubleRow), 3 terms x OC MMs.
            f16: k-step = one k-tile, OC MMs."""
            xh, xl = xt
            if fp8:
                kp = kstart  # pair index 0..KT//2-1
                g, i = (2 * kp) // WPACK, (2 * kp) % WPACK
                xsl = xh[:, 2 * kp:2 * kp + 2, :]
                xlsl = xl[:, 2 * kp:2 * kp + 2, :]
                first = kp == 0
                last = kp == KT // 2 - 1
                for oc in range(OC):
                    sl = slice(oc * 512, (oc + 1) * 512)
                    wh = wh_g[g][:, i:i + 2, sl]
                    wl = wl_g[g][:, i:i + 2, sl]
                    # xh@Wh (start), xh@Wl, then xl@Wh (stop last)
                    nc.tensor.matmul(pss[oc][:], xsl, wh,
                                     start=first, stop=False, perf_mode=DR)
                    nc.tensor.matmul(pss[oc][:], xsl, wl,
                                     start=False, stop=False, perf_mode=DR)
                    nc.tensor.matmul(pss[oc][:], xlsl, wh,
                                     start=False, stop=last, perf_mode=DR)
            else:
                k = kstart
                g, i = k // WPACK, k % WPACK
                for oc in range(OC):
                    sl = slice(oc * 512, (oc + 1) * 512)
                    nc.tensor.matmul(
                        pss[oc][:], xh[:, k, :], wh_g[g][:, i, sl],
                        start=(k == 0), stop=(k == KT - 1))

        KSTEPS = KT // 2 if fp8 else KT

        # First INTRO token tiles are interleaved k-major so the PE consumes
        # each W' k-tile INTRO*OC*terms times as it streams in from HBM,
        # hiding the W-load ramp. INTRO*OC PSUM banks stay live.
        INTRO = 3

        def main_pass(intro):
            if intro:
                ixt = [load_x(tt) for tt in range(INTRO)]
                ips = [alloc_ps(tt) for tt in range(INTRO)]
                for ks in range(KSTEPS):
                    for tt in range(INTRO):
                        mm_steps(ixt[tt], ips[tt], ks, None)
                for tt in range(INTRO):
                    flush(tt, ips[tt])
                start_tt = INTRO
            else:
                start_tt = 0
            for tt in range(start_tt, TT):
                xt = load_x(tt)
                pss = alloc_ps(tt)
                for ks in range(KSTEPS):
                    mm_steps(xt, pss, ks, None)
                flush(tt, pss)

        # Repeat structure: the all-engine barrier + PE ramp tax at each
        # For_i back edge costs ~17 us, so unroll UNROLL passes per loop
        # body; intra-body pass boundaries are barrier-free.
        UNROLL = 8
        main_pass(intro=True)
        rem = repeat - 1
        if rem >= UNROLL:
            with tc.For_i(0, rem // UNROLL, 1):
                for _ in range(UNROLL):
                    main_pass(intro=False)
        for _ in range(rem % UNROLL):
            main_pass(intro=False)

    nc.compile()
    _nc_cache[key] = nc
    return nc


def _q8(a):
    """Round to e4m3 (RNE) and return (fp8_array, fp32_residual_source)."""
    q = np.asarray(a, dtype=F8)
    return q


def _tile_x(xh2d):
    """[T_CORE, K] -> [tt, p(k), kt, j(tok)] without copy-churn."""
    return np.ascontiguousarray(
        xh2d.reshape(TT, P, KT, P).transpose(0, 3, 2, 1))


def _tile_w(WTq):
    """[K, O_CORE] -> [g, p, i, o] (k = (g*WPACK+i)*128 + p)."""
    return np.ascontiguousarray(
        WTq.reshape(KT // WPACK, WPACK, P, O_CORE).transpose(0, 2, 1, 3))


def _prep_in_maps(x, base_weight, lora_A, lora_B, bias):
    x2d = np.ascontiguousarray(x.reshape(T_FULL, K), dtype=np.float32)
    # host-side LoRA fold: W' = W + SCALE * (B @ A), shipped as W'.T
    w_full = base_weight.astype(np.float32, copy=False) + \
        SCALE * (lora_B.astype(np.float32, copy=False)
                 @ lora_A.astype(np.float32, copy=False))
    WT = np.ascontiguousarray(w_full.T, dtype=np.float32)
    bias = bias.astype(np.float32, copy=False)

    if MODE == "f8x3":
        xs = x2d * np.float32(SX)
        xh8 = _q8(xs)
        xl8 = _q8(xs - xh8.astype(np.float32))
        xhs = [_tile_x(xh8[h * T_CORE:(h + 1) * T_CORE]) for h in range(R_SPLIT)]
        xls = [_tile_x(xl8[h * T_CORE:(h + 1) * T_CORE]) for h in range(R_SPLIT)]
        Ws = WT * np.float32(SW)
        Wh8 = _q8(Ws)
        Wl8 = _q8(Ws - Wh8.astype(np.float32))
    else:
        xhs = [np.ascontiguousarray(
            x2d[h * T_CORE:(h + 1) * T_CORE]
            .reshape(TT, P, KT, P).transpose(0, 3, 2, 1), dtype=np.float16)
            for h in range(R_SPLIT)]
        Wh16 = WT.astype(np.float16)

    in_maps = []
    for h in range(R_SPLIT):
        for q in range(C_SPLIT):
            osl = slice(q * O_CORE, (q + 1) * O_CORE)
            biasb = np.ascontiguousarray(
                np.broadcast_to(bias[osl][None, :], (P, O_CORE)))
            if MODE == "f8x3":
                m = {"xbh": xhs[h], "xbl": xls[h],
                     "wth": _tile_w(np.ascontiguousarray(Wh8[:, osl])),
                     "wtl": _tile_w(np.ascontiguousarray(Wl8[:, osl])),
                     "biasb": biasb}
            else:
                m = {"xbh": xhs[h],
                     "wth": _tile_w(np.ascontiguousarray(Wh16[:, osl])),
                     "biasb": biasb}
            in_maps.append(m)
    return in_maps


def _assemble(results):
    flat = np.empty((T_FULL, D_OUT), dtype=np.float32)
    i = 0
    for h in range(R_SPLIT):
        for q in range(C_SPLIT):
            flat[h * T_CORE:(h + 1) * T_CORE,
                 q * O_CORE:(q + 1) * O_CORE] = results[i]["out"]
            i += 1
    return flat.reshape(B, S, D_OUT)


def kernel(x, base_weight, lora_A, lora_B, bias):
    x = np.asarray(x)
    base_weight = np.asarray(base_weight)
    lora_A = np.asarray(lora_A)
    lora_B = np.asarray(lora_B)
    bias = np.asarray(bias)
    nc = build_nc()
    in_maps = _prep_in_maps(x, base_weight, lora_A, lora_B, bias)
    res = run_bass_kernel_spmd(nc, in_maps, core_ids=list(range(N_CORES)))
    return _assemble(res.results)


# revision 14
# speedup vs baseline: 1.0522x; 1.0522x over previous
"""LoRA linear kernel for 8 TRN2 NeuronCores.

Computes out = x @ (base_weight + SCALE * lora_B @ lora_A).T + bias
for x [4, 2048, 4096], base_weight [4096, 4096], rank 8.

Strategy (v5):
  - The LoRA fold W' = W + SCALE*(B@A) is 0.1% of the flops -> computed on
    the HOST in numpy. The device kernel is a pure GEMM + bias.
  - Sharding: 2 token-halves x 4 d_out-quarters = 8 cores (tensor-parallel
    on d_out per the hint, plus a token split that cuts per-core x traffic).
  - fp8 3-term matmul ('f8x3'): the PE runs e4m3 DoubleRow matmuls at 2x
    rate (0.5 cyc/row).  Pure fp8 quantization error is ~3.7e-2 rel L2
    (fails the 2e-2 gate), so x and W' are each split into an fp8 high +
    fp8 residual part at the SAME power-of-2 scale:
        x*sx ~ xh + xl,  W'*sw ~ Wh + Wl
        out = [xh@Wh + xl@Wh + xh@Wl] / (sx*sw) + bias    (~1.1e-3 rel L2)
    Equal scales let all 3 terms accumulate into one fp32 PSUM bank; the
    1/(sx*sw) rescale + bias ride the flush as one DVE scalar_tensor_tensor.
    3 terms x 0.5 cyc/row = 75% of the fp16/f32r PE cycles.
  - Per core: Wh/Wl cached in SBUF as fp8 [128, 4, O_CORE] groups (8 MB).
    Main loop per 128-token tile: two 0.5 MB x DMAs (hi/lo, [128k, 32kt,
    128tok] pre-tiled on host), 3x16x2 accumulating DoubleRow matmuls into
    2 [128, 512] PSUM banks (x k-pair stationary, W' k-pair moving), DVE
    rescale+bias into fp16 out tile, 0.25 MB out DMA (host upcasts).
  - x loads issue on Sync, out stores on GpSimd (separate queues so the
    next iteration's x prefetch isn't serialized behind out-store waits).
  - The k-major INTRO interleave (hides the W'-stream ramp) runs ONCE
    before the repeat loop; repeat iterations run the plain token-major
    loop.

Sustained PE clock is ~2.0 GHz (2.4 GHz burst): fp16 floor is
2048 matmuls x 256 ns = 524 us/pass; f8x3 targets ~0.75 of that.
"""
import sys

if '/opt/trn_rl_repo' not in sys.path:
    sys.path.insert(0, '/opt/trn_rl_repo')

from contextlib import ExitStack

import numpy as np
import ml_dtypes

import concourse.bacc as bacc
import concourse.mybir as mybir
import concourse.tile as tile
from concourse.bass_utils import run_bass_kernel_spmd

SCALE = 16.0 / 8.0  # alpha / rank

MODE = "f16"        # 'f8x3' (fp8 DoubleRow 3-term) or 'f16'
SX = 16.0           # fp8 quantization scale for x  (|x*SX|  < 240)
SW = 1024.0         # fp8 quantization scale for W' (|W'*SW| < 240)
F8 = ml_dtypes.float8_e4m3

P = 128
K = 4096           # d_in (contraction)
KT = K // P        # 32 k-tiles
D_OUT = 4096
B, S = 4, 2048
T_FULL = B * S     # 8192 tokens

R_SPLIT, C_SPLIT = 2, 4
N_CORES = R_SPLIT * C_SPLIT
T_CORE = T_FULL // R_SPLIT
TT = T_CORE // P              # token tiles/core
O_CORE = D_OUT // C_SPLIT
OC = O_CORE // 512            # o-chunks of 512
WPACK = 4                     # k-tiles per W' SBUF tile

_nc_cache = {}


def build_nc(repeat=1):
    """Build the per-core Bass program. `repeat` re-runs the main loop that
    many times (first pass with the INTRO ramp, the rest in a hardware
    loop; identical results; used for slope-based HW timing)."""
    key = (MODE, repeat)
    if key in _nc_cache:
        return _nc_cache[key]
    f32 = mybir.dt.float32
    f16 = mybir.dt.float16
    f8 = mybir.dt.float8e4
    fp8 = MODE == "f8x3"
    mm_dt = f8 if fp8 else f16
    DR = mybir.MatmulPerfMode.DoubleRow if fp8 else None

    nc = bacc.Bacc(None, target_bir_lowering=False)
    # x blocks: [t_tile, p(k-within-tile), kt, j(token-within-tile)]
    xbh = nc.dram_tensor("xbh", [TT, P, KT, P], mm_dt, kind="ExternalInput")
    wth = nc.dram_tensor("wth", [KT // WPACK, P, WPACK, O_CORE], mm_dt,
                         kind="ExternalInput")
    if fp8:
        xbl = nc.dram_tensor("xbl", [TT, P, KT, P], f8, kind="ExternalInput")
        wtl = nc.dram_tensor("wtl", [KT // WPACK, P, WPACK, O_CORE], f8,
                             kind="ExternalInput")
    biasb = nc.dram_tensor("biasb", [P, O_CORE], f32, kind="ExternalInput")
    out = nc.dram_tensor("out", [T_CORE, O_CORE], f16, kind="ExternalOutput")

    with ExitStack() as ctx:
        tc = ctx.enter_context(tile.TileContext(nc))
        wpool = ctx.enter_context(tc.tile_pool(name="wpool", bufs=1))
        cpool = ctx.enter_context(tc.tile_pool(name="cpool", bufs=1))
        xpool = ctx.enter_context(tc.tile_pool(name="xpool", bufs=4))
        opool = ctx.enter_context(tc.tile_pool(name="opool", bufs=3))
        pspool = ctx.enter_context(tc.tile_pool(name="ps", bufs=4,
                                                space="PSUM"))

        # ---- main loop ----
        def load_x(tt):
            xh = xpool.tile([P, KT, P], mm_dt, name=f"xh_{tt}", tag="xh")
            nc.sync.dma_start(xh[:], xbh[tt])
            if not fp8:
                return xh, None
            xl = xpool.tile([P, KT, P], f8, name=f"xl_{tt}", tag="xl")
            nc.sync.dma_start(xl[:], xbl[tt])
            return xh, xl

        def alloc_ps(tt):
            return [pspool.tile([P, 512], f32, tag=f"ps{oc}",
                                name=f"ps_{tt}_{oc}")
                    for oc in range(OC)]

        def flush(tt, pss):
            o_t = opool.tile([P, O_CORE], f16, name=f"ot_{tt}", tag="ot")
            for oc in range(OC):
                sl = slice(oc * 512, (oc + 1) * 512)
                if fp8:
                    nc.vector.scalar_tensor_tensor(
                        o_t[:, sl], pss[oc][:], 1.0 / (SX * SW),
                        bias_t[:, sl],
                        op0=mybir.AluOpType.mult, op1=mybir.AluOpType.add)
                else:
                    nc.vector.tensor_add(o_t[:, sl], pss[oc][:],
                                         bias_t[:, sl])
            nc.gpsimd.dma_start(out[tt * P:(tt + 1) * P, :], o_t[:])

        def mm_steps(xt, pss, kstart, kstop):
            """Emit the matmuls for one k-step of one token tile.
            fp8: k-step = pair of k-tiles (DoubleRow), 3 terms x OC MMs.
            f16: k-step = one k-tile, OC MMs."""
            xh, xl = xt
            if fp8:
                kp = kstart  # pair index 0..KT//2-1
                g, i = (2 * kp) // WPACK, (2 * kp) % WPACK
                xsl = xh[:, 2 * kp:2 * kp + 2, :]
                xlsl = xl[:, 2 * kp:2 * kp + 2, :]
                first = kp == 0
                last = kp == KT // 2 - 1
                for oc in range(OC):
                    sl = slice(oc * 512, (oc + 1) * 512)
                    wh = wh_g[g][:, i:i + 2, sl]
                    wl = wl_g[g][:, i:i + 2, sl]
                    # xh@Wh (start), xh@Wl, then xl@Wh (stop last)
                    nc.tensor.matmul(pss[oc][:], xsl, wh,
                                     start=first, stop=False, perf_mode=DR)
                    nc.tensor.matmul(pss[oc][:], xsl, wl,
                                     start=False, stop=False, perf_mode=DR)
                    nc.tensor.matmul(pss[oc][:], xlsl, wh,
                                     start=False, stop=last, perf_mode=DR)
            else:
                k = kstart
                g, i = k // WPACK, k % WPACK
                for oc in range(OC):
                    sl = slice(oc * 512, (oc + 1) * 512)
                    nc.tensor.matmul(
                        pss[oc][:], xh[:, k, :], wh_g[g][:, i, sl],
                        start=(k == 0), stop=(k == KT - 1))

        KSTEPS = KT // 2 if fp8 else KT

        # First INTRO token tiles are interleaved k-major so the PE consumes
        # each W' k-tile INTRO*OC*terms times as it streams in from HBM,
        # hiding the W-load ramp. INTRO*OC PSUM banks stay live.
        INTRO = 3

        # DMA issue order = consumption order: intro x tiles first, then the
        # first W' group (the intro k-major loop needs x0-2 + wg0 to start),
        # then bias and the remaining W' stream. Cuts the pre-first-matmul
        # ramp from ~34 us to ~the first 5 MB of DMA.
        pre_x = [load_x(tt) for tt in range(INTRO)]

        wh_g, wl_g = [], []
        bias_t = None
        for g in range(KT // WPACK):
            w_g = wpool.tile([P, WPACK, O_CORE], mm_dt, tag=f"wgh{g}")
            nc.sync.dma_start(w_g[:], wth[g])
            wh_g.append(w_g)
            if fp8:
                l_g = wpool.tile([P, WPACK, O_CORE], f8, tag=f"wgl{g}")
                nc.sync.dma_start(l_g[:], wtl[g])
                wl_g.append(l_g)
            if g == 0:
                bias_t = cpool.tile([P, O_CORE], f32, tag="bias")
                nc.sync.dma_start(bias_t[:], biasb[:])

        def main_pass(intro):
            if intro:
                ixt = pre_x
                ips = [alloc_ps(tt) for tt in range(INTRO)]
                for ks in range(KSTEPS):
                    for tt in range(INTRO):
                        mm_steps(ixt[tt], ips[tt], ks, None)
                for tt in range(INTRO):
                    flush(tt, ips[tt])
                start_tt = INTRO
            else:
                start_tt = 0
            for tt in range(start_tt, TT):
                xt = load_x(tt)
                pss = alloc_ps(tt)
                for ks in range(KSTEPS):
                    mm_steps(xt, pss, ks, None)
                flush(tt, pss)

        # Repeat structure: the all-engine barrier + PE ramp tax at each
        # For_i back edge costs ~17 us, so unroll UNROLL passes per loop
        # body; intra-body pass boundaries are barrier-free.
        UNROLL = 8
        main_pass(intro=True)
        rem = repeat - 1
        if rem >= UNROLL:
            with tc.For_i(0, rem // UNROLL, 1):
                for _ in range(UNROLL):
                    main_pass(intro=False)
        for _ in range(rem % UNROLL):
            main_pass(intro=False)

    nc.compile()
    _nc_cache[key] = nc
    return nc


def _q8(a):
    """Round to e4m3 (RNE) and return (fp8_array, fp32_residual_source)."""
    q = np.asarray(a, dtype=F8)
    return q


def _tile_x(xh2d):
    """[T_CORE, K] -> [tt, p(k), kt, j(tok)] without copy-churn."""
    return np.ascontiguousarray(
        xh2d.reshape(TT, P, KT, P).transpose(0, 3, 2, 1))


def _tile_w(WTq):
    """[K, O_CORE] -> [g, p, i, o] (k = (g*WPACK+i)*128 + p)."""
    return np.ascontiguousarray(
        WTq.reshape(KT // WPACK, WPACK, P, O_CORE).transpose(0, 2, 1, 3))


def _prep_in_maps(x, base_weight, lora_A, lora_B, bias):
    x2d = np.ascontiguousarray(x.reshape(T_FULL, K), dtype=np.float32)
    # host-side LoRA fold: W' = W + SCALE * (B @ A), shipped as W'.T
    w_full = base_weight.astype(np.float32, copy=False) + \
        SCALE * (lora_B.astype(np.float32, copy=False)
                 @ lora_A.astype(np.float32, copy=False))
    WT = np.ascontiguousarray(w_full.T, dtype=np.float32)
    bias = bias.astype(np.float32, copy=False)

    if MODE == "f8x3":
        xs = x2d * np.float32(SX)
        xh8 = _q8(xs)
        xl8 = _q8(xs - xh8.astype(np.float32))
        xhs = [_tile_x(xh8[h * T_CORE:(h + 1) * T_CORE]) for h in range(R_SPLIT)]
        xls = [_tile_x(xl8[h * T_CORE:(h + 1) * T_CORE]) for h in range(R_SPLIT)]
        Ws = WT * np.float32(SW)
        Wh8 = _q8(Ws)
        Wl8 = _q8(Ws - Wh8.astype(np.float32))
    else:
        xhs = [np.ascontiguousarray(
            x2d[h * T_CORE:(h + 1) * T_CORE]
            .reshape(TT, P, KT, P).transpose(0, 3, 2, 1), dtype=np.float16)
            for h in range(R_SPLIT)]
        Wh16 = WT.astype(np.float16)

    in_maps = []
    for h in range(R_SPLIT):
        for q in range(C_SPLIT):
            osl = slice(q * O_CORE, (q + 1) * O_CORE)
            biasb = np.ascontiguousarray(
                np.broadcast_to(bias[osl][None, :], (P, O_CORE)))
            if MODE == "f8x3":
                m = {"xbh": xhs[h], "xbl": xls[h],
                     "wth": _tile_w(np.ascontiguousarray(Wh8[:, osl])),
                     "wtl": _tile_w(np.ascontiguousarray(Wl8[:, osl])),
                     "biasb": biasb}
            else:
                m = {"xbh": xhs[h],
                     "wth": _tile_w(np.ascontiguousarray(Wh16[:, osl])),
                     "biasb": biasb}
            in_maps.append(m)
    return in_maps


def _assemble(results):
    flat = np.empty((T_FULL, D_OUT), dtype=np.float32)
    i = 0
    for h in range(R_SPLIT):
        for q in range(C_SPLIT):
            flat[h * T_CORE:(h + 1) * T_CORE,
                 q * O_CORE:(q + 1) * O_CORE] = results[i]["out"]
            i += 1
    return flat.reshape(B, S, D_OUT)


def kernel(x, base_weight, lora_A, lora_B, bias):
    x = np.asarray(x)
    base_weight = np.asarray(base_weight)
    lora_A = np.asarray(lora_A)
    lora_B = np.asarray(lora_B)
    bias = np.asarray(bias)
    nc = build_nc()
    in_maps = _prep_in_maps(x, base_weight, lora_A, lora_B, bias)
    res = run_bass_kernel_spmd(nc, in_maps, core_ids=list(range(N_CORES)))
    return _assemble(res.results)
